# revision 1
# baseline (speedup 1.0000x reference)
"""KAN layer (identity edges) Trainium2 kernel.

output[b, o] = sum_i x[b, i]  for all o  -- row-sum broadcast to (B, 1024).

Data-parallel over 8 NeuronCores: each core gets 8192 rows of x
(65536 x 1024 f32), computes row sums on the Vector engine, broadcasts
across the feature dim on-chip, and DMAs the full (8192, 1024) shard out.

Layout: partition p owns 64 consecutive DRAM rows (rearrange
"(p n) d -> p n d"), so each DMA moves R*4KB contiguous bytes per
partition.

Perf notes (HW-traced):
- Loads go on the SP HWDGE ring, stores on the ACT HWDGE ring; the two
  rings share the ~435 GB/s SBUF-AXI/SDMA budget via per-packet
  round-robin, so a 1:1 queue split keeps read and write streams
  balanced (64 MB/core total -> ~147 us DMA floor uncontended).
- The first/last iterations use smaller tiles (ramp) so the write
  stream starts earlier and the tail write runs shorter solo.
- Compute (reduce ~8.7us + bcast copy ~4.4us per 8-row tile on DVE)
  stays fully hidden under DMA.
"""

import numpy as np

import concourse.tile as tile
from concourse import bacc, mybir
from concourse.bass_utils import run_bass_kernel_spmd

N_CORES = 8
BATCH = 65536
FEAT = 1024
ROWS = BATCH // N_CORES        # 8192 rows per core
P = 128                        # SBUF partitions
ROWS_PER_PART = ROWS // P      # 64 consecutive rows owned by each partition

R_SCHED = (2, 2, 4, 8, 8, 8, 8, 8, 8, 4, 4)
IN_BUFS = 3
OUT_BUFS = 3

_nc_cache = []


def _build():
    assert sum(R_SCHED) == ROWS_PER_PART
    nc = bacc.Bacc()
    x = nc.declare_dram_parameter("x", [ROWS, FEAT], mybir.dt.float32, isOutput=False)
    y = nc.declare_dram_parameter("y", [ROWS, FEAT], mybir.dt.float32, isOutput=True)
    xv = x[:, :].rearrange("(p n) d -> p n d", p=P)
    yv = y[:, :].rearrange("(p n) d -> p n d", p=P)

    with tile.TileContext(nc) as tc:
        with (
            tc.tile_pool(name="inp", bufs=IN_BUFS) as inp,
            tc.tile_pool(name="outp", bufs=OUT_BUFS) as outp,
            tc.tile_pool(name="sums", bufs=4) as sums_pool,
        ):
            row = 0
            for r in R_SCHED:
                t = inp.tile([P, r, FEAT], mybir.dt.float32, tag="in")
                nc.sync.dma_start(out=t[:, :, :], in_=xv[:, row : row + r, :])

                s = sums_pool.tile([P, r], mybir.dt.float32, tag="s")
                nc.vector.reduce_sum(
                    out=s[:, :], in_=t[:, :, :], axis=mybir.AxisListType.X
                )

                o = outp.tile([P, r, FEAT], mybir.dt.float32, tag="out")
                nc.vector.tensor_copy(
                    out=o[:, :, :], in_=s[:, :].to_broadcast([P, r, FEAT])
                )
                nc.scalar.dma_start(out=yv[:, row : row + r, :], in_=o[:, :, :])
                row += r
    nc.finalize()
    return nc


def _get_nc():
    if not _nc_cache:
        _nc_cache.append(_build())
    return _nc_cache[0]


def kernel(x: np.ndarray) -> np.ndarray:
    nc = _get_nc()
    x = np.ascontiguousarray(np.asarray(x), dtype=np.float32)
    shards = np.split(x, N_CORES, axis=0)
    in_maps = [{"x": s} for s in shards]
    res = run_bass_kernel_spmd(nc, in_maps, list(range(N_CORES)))
    return np.concatenate([res.results[i]["y"] for i in range(N_CORES)], axis=0)



# revision 2
# speedup vs baseline: 1.5832x; 1.5832x over previous
"""KAN layer (identity edges) Trainium2 kernel.

output[b, o] = sum_i x[b, i]  for all o  -- row-sum broadcast to (B, 1024).

The output is rank-1 along the feature dim (every row is a single scalar
repeated 1024x), so the device kernel computes ONLY the row sums
(the actual reduction work) and the host reconstructs the broadcast as a
stride-0 view during the unshard step -- exactly what the reference's
jnp.broadcast_to does (a free view, no data movement).

Data-parallel over 8 NeuronCores: each core streams its 8192x1024 f32
shard of x from HBM (32 MiB), reduces rows on the Vector engine, and
writes back just the 8192 sums (32 KiB).  HBM traffic per core drops
from 64 MiB (in+out) to ~32 MiB, i.e. the read-only roofline.

Layout: partition p owns 64 consecutive DRAM rows (rearrange
"(p n) d -> p n d"), so each load moves r*4KB contiguous bytes per
partition.  Loads alternate between the two HWDGE rings (SP via
nc.sync, ACT via nc.scalar) so per-DMA issue/completion gaps on one
ring are hidden by the other; the single tiny sums store goes last.
The tail tiles shrink (8,...,4,2,2) so the final reduce + store after
the last load is only ~3-4 us.
"""

import numpy as np

import concourse.tile as tile
from concourse import bacc, mybir
from concourse.bass_utils import run_bass_kernel_spmd

N_CORES = 8
BATCH = 65536
FEAT = 1024
ROWS = BATCH // N_CORES        # 8192 rows per core
P = 128                        # SBUF partitions
ROWS_PER_PART = ROWS // P      # 64 consecutive rows owned by each partition

R_SCHED = (8, 8, 8, 8, 8, 8, 8, 4, 2, 2)
IN_BUFS = 4

_nc_cache = []


def _build():
    assert sum(R_SCHED) == ROWS_PER_PART
    nc = bacc.Bacc()
    x = nc.declare_dram_parameter("x", [ROWS, FEAT], mybir.dt.float32, isOutput=False)
    y = nc.declare_dram_parameter(
        "y", [P, ROWS_PER_PART], mybir.dt.float32, isOutput=True
    )
    xv = x[:, :].rearrange("(p n) d -> p n d", p=P)

    with tile.TileContext(nc) as tc:
        with (
            tc.tile_pool(name="inp", bufs=IN_BUFS) as inp,
            tc.tile_pool(name="sums", bufs=1) as sums_pool,
        ):
            s_all = sums_pool.tile([P, ROWS_PER_PART], mybir.dt.float32, tag="s")
            row = 0
            for i, r in enumerate(R_SCHED):
                t = inp.tile([P, r, FEAT], mybir.dt.float32, tag="in")
                eng = nc.sync if i % 2 == 0 else nc.scalar
                eng.dma_start(out=t[:, :, :], in_=xv[:, row : row + r, :])
                nc.vector.reduce_sum(
                    out=s_all[:, row : row + r],
                    in_=t[:, :, :],
                    axis=mybir.AxisListType.X,
                )
                row += r
            nc.sync.dma_start(out=y[:, :], in_=s_all[:, :])
    nc.finalize()
    return nc


def _get_nc():
    if not _nc_cache:
        _nc_cache.append(_build())
    return _nc_cache[0]


def kernel(x: np.ndarray) -> np.ndarray:
    nc = _get_nc()
    x = np.ascontiguousarray(np.asarray(x), dtype=np.float32)
    shards = np.split(x, N_CORES, axis=0)
    in_maps = [{"x": s} for s in shards]
    res = run_bass_kernel_spmd(nc, in_maps, list(range(N_CORES)))
    sums = np.concatenate(
        [res.results[i]["y"].reshape(ROWS) for i in range(N_CORES)], axis=0
    )
    return np.broadcast_to(sums[:, None], (BATCH, FEAT))


# revision 4
# speedup vs baseline: 1.6952x; 1.0707x over previous
"""KAN layer (identity edges) Trainium2 kernel.

output[b, o] = sum_i x[b, i]  for all o  -- row-sum broadcast to (B, 1024).

The output is rank-1 along the feature dim (every row is a single scalar
repeated 1024x), so the device kernel computes ONLY the row sums
(the actual reduction work) and the host reconstructs the broadcast as a
stride-0 view during the unshard step -- exactly what the reference's
jnp.broadcast_to does (a free view, no data movement).

Data-parallel over 8 NeuronCores: each core streams its 8192x1024 f32
shard of x from HBM (32 MiB), reduces rows on the Vector engine, and
writes back just the 8192 sums (32 KiB).  HBM traffic per core drops
from 64 MiB (in+out) to ~32 MiB, i.e. the read-only roofline.

Layout: partition p owns 64 consecutive DRAM rows (rearrange
"(p n) d -> p n d"), so each load moves r*4KB contiguous bytes per
partition.  Loads alternate between the two HWDGE rings (SP via
nc.sync, ACT via nc.scalar) so per-DMA issue/completion gaps on one
ring are hidden by the other; the single tiny sums store goes last.
The tail tiles shrink (8,...,4,2,2) so the final reduce + store after
the last load is only ~3-4 us.
"""

import numpy as np

import concourse.tile as tile
from concourse import bacc, mybir
from concourse.bass_utils import run_bass_kernel_spmd

N_CORES = 8
BATCH = 65536
FEAT = 1024
ROWS = BATCH // N_CORES        # 8192 rows per core
P = 128                        # SBUF partitions
ROWS_PER_PART = ROWS // P      # 64 consecutive rows owned by each partition

R_SCHED = (8, 8, 8, 8, 8, 8, 8, 4, 2, 2)
IN_BUFS = 5

_nc_cache = []


def _build():
    assert sum(R_SCHED) == ROWS_PER_PART
    nc = bacc.Bacc()
    x = nc.declare_dram_parameter("x", [ROWS, FEAT], mybir.dt.float32, isOutput=False)
    y = nc.declare_dram_parameter(
        "y", [P, ROWS_PER_PART], mybir.dt.float32, isOutput=True
    )
    xv = x[:, :].rearrange("(p n) d -> p n d", p=P)

    with tile.TileContext(nc) as tc:
        with (
            tc.tile_pool(name="inp", bufs=IN_BUFS) as inp,
            tc.tile_pool(name="sums", bufs=1) as sums_pool,
            tc.tile_pool(name="scratch", bufs=1) as scratch_pool,
        ):
            s_all = sums_pool.tile([P, ROWS_PER_PART], mybir.dt.float32, tag="s")
            # ACT's activation op must write a full-size elementwise output
            # alongside the accumulator; all ACT tiles share this scratch
            # (ACT executes its own ops in order, so reuse is safe).
            scr = scratch_pool.tile([P, max(R_SCHED), FEAT], mybir.dt.float32, tag="scr")
            row = 0
            for i, r in enumerate(R_SCHED):
                t = inp.tile([P, r, FEAT], mybir.dt.float32, tag="in")
                eng = nc.sync if i % 2 == 0 else nc.scalar
                eng.dma_start(out=t[:, :, :], in_=xv[:, row : row + r, :])
                if i % 2 == 0:
                    # whole-tile row reduce on the Vector engine
                    nc.vector.reduce_sum(
                        out=s_all[:, row : row + r],
                        in_=t[:, :, :],
                        axis=mybir.AxisListType.X,
                    )
                else:
                    # per-row reduce on the Scalar (ACT) engine via the
                    # activation accumulator, running parallel to DVE
                    for j in range(r):
                        nc.scalar.activation(
                            out=scr[:, j, :],
                            in_=t[:, j, :],
                            func=mybir.ActivationFunctionType.Copy,
                            accum_out=s_all[:, row + j : row + j + 1],
                        )
                row += r
            nc.sync.dma_start(out=y[:, :], in_=s_all[:, :])
    nc.finalize()
    return nc


def _get_nc():
    if not _nc_cache:
        _nc_cache.append(_build())
    return _nc_cache[0]


def kernel(x: np.ndarray) -> np.ndarray:
    nc = _get_nc()
    x = np.ascontiguousarray(np.asarray(x), dtype=np.float32)
    shards = np.split(x, N_CORES, axis=0)
    in_maps = [{"x": s} for s in shards]
    res = run_bass_kernel_spmd(nc, in_maps, list(range(N_CORES)))
    sums = np.concatenate(
        [res.results[i]["y"].reshape(ROWS) for i in range(N_CORES)], axis=0
    )
    return np.broadcast_to(sums[:, None], (BATCH, FEAT))


# revision 5
# speedup vs baseline: 1.8227x; 1.0752x over previous
"""KAN layer (identity edges) Trainium2 kernel.

output[b, o] = sum_i x[b, i]  for all o  -- row-sum broadcast to (B, 1024).

Two structural optimizations over a naive full-output kernel:

1. The output is rank-1 along the feature dim (every row is one scalar
   repeated 1024x), so the device computes ONLY the row sums and the
   host reconstructs the broadcast as a stride-0 view during unshard --
   exactly the reference's jnp.broadcast_to (a free view).  This removes
   the entire 256 MB output write from HBM.

2. The row-sum is extremely error-tolerant (1024-term dot with ones,
   accumulated in fp32 on the Vector engine), so the host casts x to
   fp16 before upload.  This halves HBM read traffic; measured L2
   relative error ~2e-4 (vs 5e-7 for f32), far inside the 2e-2 gate.
   fp16 is preferred over bf16 for its 10-bit mantissa.

Data-parallel over 8 NeuronCores: each core streams its 8192x1024 fp16
shard (16 MiB), reduces rows (fp32 accumulate), writes back 8192 f32
sums (32 KiB).

Layout: partition p owns 64 consecutive DRAM rows (rearrange
"(p n) d -> p n d"); loads alternate between the two HWDGE rings
(SP/nc.sync, ACT/nc.scalar) to hide per-DMA issue gaps.
"""

import numpy as np

import concourse.tile as tile
from concourse import bacc, mybir
from concourse.bass_utils import run_bass_kernel_spmd

N_CORES = 8
BATCH = 65536
FEAT = 1024
ROWS = BATCH // N_CORES        # 8192 rows per core
P = 128                        # SBUF partitions
ROWS_PER_PART = ROWS // P      # 64 consecutive rows owned by each partition

R_SCHED = (8, 8, 8, 8, 8, 8, 8, 4, 2, 2)
IN_BUFS = 8

_nc_cache = []


def _build():
    assert sum(R_SCHED) == ROWS_PER_PART
    nc = bacc.Bacc()
    x = nc.declare_dram_parameter("x", [ROWS, FEAT], mybir.dt.float16, isOutput=False)
    y = nc.declare_dram_parameter(
        "y", [P, ROWS_PER_PART], mybir.dt.float32, isOutput=True
    )
    xv = x[:, :].rearrange("(p n) d -> p n d", p=P)

    with tile.TileContext(nc) as tc:
        with (
            tc.tile_pool(name="inp", bufs=IN_BUFS) as inp,
            tc.tile_pool(name="sums", bufs=1) as sums_pool,
        ):
            s_all = sums_pool.tile([P, ROWS_PER_PART], mybir.dt.float32, tag="s")
            row = 0
            for i, r in enumerate(R_SCHED):
                t = inp.tile([P, r, FEAT], mybir.dt.float16, tag="in")
                eng = nc.sync if i % 2 == 0 else nc.scalar
                eng.dma_start(out=t[:, :, :], in_=xv[:, row : row + r, :])
                nc.vector.reduce_sum(
                    out=s_all[:, row : row + r],
                    in_=t[:, :, :],
                    axis=mybir.AxisListType.X,
                )
                row += r
            nc.sync.dma_start(out=y[:, :], in_=s_all[:, :])
    nc.finalize()
    return nc


def _get_nc():
    if not _nc_cache:
        _nc_cache.append(_build())
    return _nc_cache[0]


def kernel(x: np.ndarray) -> np.ndarray:
    nc = _get_nc()
    xh = np.ascontiguousarray(np.asarray(x)).astype(np.float16)
    shards = np.split(xh, N_CORES, axis=0)
    in_maps = [{"x": s} for s in shards]
    res = run_bass_kernel_spmd(nc, in_maps, list(range(N_CORES)))
    sums = np.concatenate(
        [res.results[i]["y"].reshape(ROWS) for i in range(N_CORES)], axis=0
    )
    return np.broadcast_to(sums[:, None], (BATCH, FEAT))


# revision 6
# speedup vs baseline: 2.2752x; 1.2483x over previous
"""KAN layer (identity edges) Trainium2 kernel.

output[b, o] = sum_i x[b, i]  for all o  -- row-sum broadcast to (B, 1024).

Structural optimizations over a naive full-output kernel:

1. Rank-1 output: the device computes ONLY the row sums; the host
   reconstructs the broadcast as a stride-0 view during unshard (the
   reference's own jnp.broadcast_to is the same free view).  Removes the
   entire 256 MB output write from HBM.

2. fp16 ingest: the row-sum tolerates quantization easily (1024-term
   sum, fp32 accumulation; measured L2 rel err ~3e-4 vs the 2e-2 gate),
   so the host casts x to fp16 before upload, halving HBM read traffic
   to 16 MiB/core.

3. Compute keeps pace with the ~470 GB/s load stream by splitting rows
   between two engines, chosen from HW-probed rates:
     - DVE: within-row halving tensor_tensor adds (fp16 2x_1P packed
       mode, 2 elem/cycle) down to 128 wide, then a 1x reduce_sum with
       f32 output: ~0.63 us/row vs 1.06 us/row for a plain reduce.
     - ACT: activation(Copy) with accum_out f32: ~1.41 us/row, runs in
       parallel on early tiles.
   GpSimd tensor ops measured ~2x slower than DVE -- not used.

4. Ring discipline: the ACT sequencer issues ring-B loads, so ring B
   carries only tiles 1 and 3, both issued before any ACT compute op;
   ACT never delays a load.  Tail tiles shrink (8,4,2,2) so the final
   reduce+store after the last load is ~3 us.
"""

import numpy as np

import concourse.tile as tile
from concourse import bacc, mybir
from concourse.bass_utils import run_bass_kernel_spmd

N_CORES = 8
BATCH = 65536
FEAT = 1024
ROWS = BATCH // N_CORES        # 8192 rows per core
P = 128                        # SBUF partitions
ROWS_PER_PART = ROWS // P      # 64 consecutive rows owned by each partition

# tile sizes in rows-per-partition; ring B (scalar/ACT) loads tiles 1 and 3
R_SCHED = (16, 16, 16, 8, 4, 2, 2)
RING_B = (1, 3)
# rows per tile reduced by ACT (from the front); DVE takes the rest
ACT_ROWS = (5, 5, 5, 3, 2, 0, 0)
IN_BUFS = 5
F16 = mybir.dt.float16
F32 = mybir.dt.float32

_nc_cache = []


def _dve_tree_reduce(nc, t, r0, r1, h1, h2, h3, s_all, row):
    """Row sums of t[:, r0:r1, :1024] -> s_all[:, row+r0 : row+r1] via
    fp16 halving adds (2x packed) + final 128-wide 1x reduce (f32 out)."""
    n = r1 - r0
    if n <= 0:
        return
    if n == 1:
        # plain per-row reduce; tree overhead isn't worth it for 1 row
        nc.vector.reduce_sum(
            out=s_all[:, row + r0 : row + r1],
            in_=t[:, r0, :],
            axis=mybir.AxisListType.X,
        )
        return
    nc.vector.tensor_add(
        out=h1[:, 0:n, :], in0=t[:, r0:r1, 0:512], in1=t[:, r0:r1, 512:1024]
    )
    nc.vector.tensor_add(
        out=h2[:, 0:n, :], in0=h1[:, 0:n, 0:256], in1=h1[:, 0:n, 256:512]
    )
    nc.vector.tensor_add(
        out=h3[:, 0:n, :], in0=h2[:, 0:n, 0:128], in1=h2[:, 0:n, 128:256]
    )
    nc.vector.reduce_sum(
        out=s_all[:, row + r0 : row + r1],
        in_=h3[:, 0:n, :],
        axis=mybir.AxisListType.X,
    )


def _build():
    assert sum(R_SCHED) == ROWS_PER_PART
    nc = bacc.Bacc()
    x = nc.declare_dram_parameter("x", [ROWS, FEAT], F16, isOutput=False)
    y = nc.declare_dram_parameter("y", [P, ROWS_PER_PART], F32, isOutput=True)
    xv = x[:, :].rearrange("(p n) d -> p n d", p=P)

    max_r = max(R_SCHED)
    max_act = max(ACT_ROWS)

    with tile.TileContext(nc) as tc:
        with (
            tc.tile_pool(name="inp", bufs=IN_BUFS) as inp,
            tc.tile_pool(name="sums", bufs=1) as sums_pool,
            tc.tile_pool(name="tree", bufs=1) as tree_pool,
            tc.tile_pool(name="scr", bufs=1) as scr_pool,
        ):
            s_all = sums_pool.tile([P, ROWS_PER_PART], F32, tag="s")
            h1 = tree_pool.tile([P, max_r, 512], F16, tag="h1")
            h2 = tree_pool.tile([P, max_r, 256], F16, tag="h2")
            h3 = tree_pool.tile([P, max_r, 128], F16, tag="h3")
            scr = scr_pool.tile([P, max_act, FEAT], F16, tag="scr")

            # --- emit loads first: ring B's two issues must precede any
            # ACT compute in the ACT sequencer's FIFO stream.
            tiles = []
            row = 0
            rows_of = []
            for i, r in enumerate(R_SCHED):
                t = inp.tile([P, r, FEAT], F16, tag="in")
                eng = nc.scalar if i in RING_B else nc.sync
                eng.dma_start(out=t[:, :, :], in_=xv[:, row : row + r, :])
                tiles.append(t)
                rows_of.append(row)
                row += r

            # --- compute: ACT takes the first ACT_ROWS[i] rows of each
            # tile (per-row activation+accum), DVE tree-reduces the rest.
            for i, r in enumerate(R_SCHED):
                t = tiles[i]
                row = rows_of[i]
                a = ACT_ROWS[i]
                for j in range(a):
                    nc.scalar.activation(
                        out=scr[:, j % max_act, :],
                        in_=t[:, j, :],
                        func=mybir.ActivationFunctionType.Copy,
                        accum_out=s_all[:, row + j : row + j + 1],
                    )
                _dve_tree_reduce(nc, t, a, r, h1, h2, h3, s_all, row)

            nc.scalar.dma_start(out=y[:, :], in_=s_all[:, :])
    nc.finalize()
    return nc


def _get_nc():
    if not _nc_cache:
        _nc_cache.append(_build())
    return _nc_cache[0]


def kernel(x: np.ndarray) -> np.ndarray:
    nc = _get_nc()
    xh = np.ascontiguousarray(np.asarray(x)).astype(np.float16)
    shards = np.split(xh, N_CORES, axis=0)
    in_maps = [{"x": s} for s in shards]
    res = run_bass_kernel_spmd(nc, in_maps, list(range(N_CORES)))
    sums = np.concatenate(
        [res.results[i]["y"].reshape(ROWS) for i in range(N_CORES)], axis=0
    )
    return np.broadcast_to(sums[:, None], (BATCH, FEAT))


# revision 9
# speedup vs baseline: 2.3143x; 1.0172x over previous
"""KAN layer (identity edges) Trainium2 kernel.

output[b, o] = sum_i x[b, i]  for all o  -- row-sum broadcast to (B, 1024).

Structural optimizations over a naive full-output kernel:

1. Rank-1 output: the device computes ONLY the row sums; the host
   reconstructs the broadcast as a stride-0 view during unshard (the
   reference's own jnp.broadcast_to is the same free view).  Removes the
   entire 256 MB output write from HBM.

2. fp16 ingest: the row-sum tolerates quantization easily (1024-term
   sum, fp32 accumulation; measured L2 rel err ~5e-4 vs the 2e-2 gate),
   so the host casts x to fp16 before upload, halving HBM read traffic
   to 16 MiB/core.

3. Three compute engines keep pace with the ~450 GB/s load stream
   (HW-probed rates):
     - DVE: within-row halving tensor_tensor adds (fp16 2x_1P packed
       mode, 2 elem/cycle) down to 128 wide + 1x reduce_sum (f32 out).
     - ACT: activation(Copy) with f32 accum_out, ~1.4 us/row, on a few
       rows of early/mid tiles.
     - GpSimd: stage-1 halving add for two mid tiles; DVE finishes them
       (the cheap 3/4 of the tree) later, so the DVE FIFO never blocks
       waiting on GpSimd.

4. Schedule: small ramp tiles (2,2,4,4) so the first data lands ~3 us
   after DMA start and compute begins immediately; both HWDGE rings
   carry equal bytes; ACT's ring-B load issues all precede its compute
   ops in its FIFO; tail tiles are 2 rows so the post-load tail is
   ~3 us.  Buffer-slot collisions with ACT-owned tiles are avoided
   (IN_BUFS=9 chosen so no scalar-issued load waits on an ACT-consumed
   buffer -- that would deadlock the ACT sequencer).
"""

import numpy as np

import concourse.tile as tile
from concourse import bacc, mybir
from concourse.bass_utils import run_bass_kernel_spmd

N_CORES = 8
BATCH = 65536
FEAT = 1024
ROWS = BATCH // N_CORES        # 8192 rows per core
P = 128                        # SBUF partitions
ROWS_PER_PART = ROWS // P      # 64 consecutive rows owned by each partition

R_SCHED = (2, 2, 4, 4, 8, 8, 8, 8, 8, 8, 2, 2)
RING_B = (1, 3, 5, 7, 9, 11)           # loads issued by the scalar (ACT) engine
ACT_ROWS = (0, 2, 0, 1, 0, 2, 0, 2, 2, 2, 0, 0)  # rows from tile front on ACT
GPS_TILES = (4, 6)                     # GpSimd does stage-1; DVE finishes late
IN_BUFS = 9
F16 = mybir.dt.float16
F32 = mybir.dt.float32

_nc_cache = []


def _dve_tree(nc, t, r0, r1, h1, h2, h3, s_all, lo):
    """Row sums of t[:, r0:r1, :1024] -> s_all[:, lo:lo+(r1-r0)] on DVE."""
    n = r1 - r0
    if n == 1:
        nc.vector.reduce_sum(
            out=s_all[:, lo : lo + 1], in_=t[:, r0, :], axis=mybir.AxisListType.X
        )
        return
    nc.vector.tensor_add(
        out=h1[:, 0:n, :], in0=t[:, r0:r1, 0:512], in1=t[:, r0:r1, 512:1024]
    )
    nc.vector.tensor_add(
        out=h2[:, 0:n, :], in0=h1[:, 0:n, 0:256], in1=h1[:, 0:n, 256:512]
    )
    nc.vector.tensor_add(
        out=h3[:, 0:n, :], in0=h2[:, 0:n, 0:128], in1=h2[:, 0:n, 128:256]
    )
    nc.vector.reduce_sum(
        out=s_all[:, lo : lo + n], in_=h3[:, 0:n, :], axis=mybir.AxisListType.X
    )


def _dve_residue(nc, g, n, h2, h3, s_all, lo):
    """Finish a GpSimd-halved tile: g[:, 0:n, 0:512] -> sums."""
    nc.vector.tensor_add(
        out=h2[:, 0:n, :], in0=g[:, 0:n, 0:256], in1=g[:, 0:n, 256:512]
    )
    nc.vector.tensor_add(
        out=h3[:, 0:n, :], in0=h2[:, 0:n, 0:128], in1=h2[:, 0:n, 128:256]
    )
    nc.vector.reduce_sum(
        out=s_all[:, lo : lo + n], in_=h3[:, 0:n, :], axis=mybir.AxisListType.X
    )


def _build():
    assert sum(R_SCHED) == ROWS_PER_PART
    nc = bacc.Bacc()
    x = nc.declare_dram_parameter("x", [ROWS, FEAT], F16, isOutput=False)
    y = nc.declare_dram_parameter("y", [P, ROWS_PER_PART], F32, isOutput=True)
    xv = x[:, :].rearrange("(p n) d -> p n d", p=P)

    max_r = max(R_SCHED)
    max_act = 2

    with tile.TileContext(nc) as tc:
        with (
            tc.tile_pool(name="inp", bufs=IN_BUFS) as inp,
            tc.tile_pool(name="sums", bufs=1) as sums_pool,
            tc.tile_pool(name="tree", bufs=1) as tree_pool,
            tc.tile_pool(name="gbuf", bufs=len(GPS_TILES)) as g_pool,
            tc.tile_pool(name="scr", bufs=1) as scr_pool,
        ):
            s_all = sums_pool.tile([P, ROWS_PER_PART], F32, tag="s")
            h1 = tree_pool.tile([P, max_r, 512], F16, tag="h1")
            h2 = tree_pool.tile([P, max_r, 256], F16, tag="h2")
            h3 = tree_pool.tile([P, max_r, 128], F16, tag="h3")
            scr = scr_pool.tile([P, max_act, FEAT], F16, tag="scr")

            # --- all loads first (ring issues precede any compute in
            # each sequencer's FIFO)
            tiles, rows_of = [], []
            row = 0
            for i, r in enumerate(R_SCHED):
                t = inp.tile([P, r, FEAT], F16, tag="in")
                eng = nc.scalar if i in RING_B else nc.sync
                eng.dma_start(out=t[:, :, :], in_=xv[:, row : row + r, :])
                tiles.append(t)
                rows_of.append(row)
                row += r

            # --- ACT rows (front rows of its tiles, ascending)
            for i, r in enumerate(R_SCHED):
                for j in range(ACT_ROWS[i]):
                    nc.scalar.activation(
                        out=scr[:, j % max_act, :],
                        in_=tiles[i][:, j, :],
                        func=mybir.ActivationFunctionType.Copy,
                        accum_out=s_all[:, rows_of[i] + j : rows_of[i] + j + 1],
                    )

            # --- GpSimd stage-1 halving for its tiles
            gbufs = {}
            for i in GPS_TILES:
                r = R_SCHED[i]
                g = g_pool.tile([P, max_r, 512], F16, tag="g")
                nc.gpsimd.tensor_add(
                    out=g[:, 0:r, :],
                    in0=tiles[i][:, :, 0:512],
                    in1=tiles[i][:, :, 512:1024],
                )
                gbufs[i] = g

            # --- DVE trees: own tiles 0..9 first, then GPS residues,
            # then the tail tiles (which land last anyway)
            tail = (len(R_SCHED) - 2, len(R_SCHED) - 1)
            for i, r in enumerate(R_SCHED):
                if i in GPS_TILES or i in tail:
                    continue
                a = ACT_ROWS[i]
                if r - a > 0:
                    _dve_tree(nc, tiles[i], a, r, h1, h2, h3, s_all, rows_of[i] + a)
            for i in GPS_TILES:
                _dve_residue(nc, gbufs[i], R_SCHED[i], h2, h3, s_all, rows_of[i])
            for i in tail:
                _dve_tree(nc, tiles[i], 0, R_SCHED[i], h1, h2, h3, s_all, rows_of[i])

            nc.scalar.dma_start(out=y[:, :], in_=s_all[:, :])
    nc.finalize()
    return nc


def _get_nc():
    if not _nc_cache:
        _nc_cache.append(_build())
    return _nc_cache[0]


def kernel(x: np.ndarray) -> np.ndarray:
    nc = _get_nc()
    xh = np.ascontiguousarray(np.asarray(x)).astype(np.float16)
    shards = np.split(xh, N_CORES, axis=0)
    in_maps = [{"x": s} for s in shards]
    res = run_bass_kernel_spmd(nc, in_maps, list(range(N_CORES)))
    sums = np.concatenate(
        [res.results[i]["y"].reshape(ROWS) for i in range(N_CORES)], axis=0
    )
    return np.broadcast_to(sums[:, None], (BATCH, FEAT))


# revision 11
# speedup vs baseline: 2.4443x; 1.0562x over previous
"""KAN layer (identity edges) Trainium2 kernel.

output[b, o] = sum_i x[b, i]  for all o  -- row-sum broadcast to (B, 1024).

Structural optimizations over a naive full-output kernel:

1. Rank-1 output: the device computes ONLY the row sums; the host
   reconstructs the broadcast as a stride-0 view during unshard (the
   reference's own jnp.broadcast_to is the same free view).  Removes the
   entire 256 MB output write from HBM.

2. fp16 ingest: the row-sum tolerates quantization easily (1024-term
   sum, fp32 accumulation; measured L2 rel err ~5e-4 vs the 2e-2 gate),
   so the host casts x to fp16 before upload, halving HBM read traffic
   to 16 MiB/core.

3. Three compute engines keep pace with the ~450 GB/s load stream
   (HW-probed rates):
     - DVE: within-row halving tensor_tensor adds (fp16 2x_1P packed
       mode, 2 elem/cycle) down to 128 wide + 1x reduce_sum (f32 out).
     - ACT: activation(Copy) with f32 accum_out, ~1.4 us/row, on the
       front rows of most tiles (~20 of 64 rows).
   GpSimd is deliberately NOT used: its tensor ops running concurrently
   with DVE packed ops degrade DVE ~5x (HW-traced SBUF interference).

4. Schedule: small ramp tiles (2,2,4,4) so the first data lands ~3 us
   after DMA start and compute begins immediately; both HWDGE rings
   carry equal bytes; ACT's ring-B load issues all precede its compute
   ops in its FIFO; tail tiles are 2 rows so the post-load tail is
   ~3 us.  Buffer-slot collisions with ACT-owned tiles are avoided
   (IN_BUFS=9 chosen so no scalar-issued load waits on an ACT-consumed
   buffer -- that would deadlock the ACT sequencer).
"""

import numpy as np

import concourse.tile as tile
from concourse import bacc, mybir
from concourse.bass_utils import run_bass_kernel_spmd

N_CORES = 8
BATCH = 65536
FEAT = 1024
ROWS = BATCH // N_CORES        # 8192 rows per core
P = 128                        # SBUF partitions
ROWS_PER_PART = ROWS // P      # 64 consecutive rows owned by each partition

R_SCHED = (2, 2, 4, 4, 8, 8, 8, 8, 8, 8, 2, 2)
RING_B = (1, 3, 5, 7, 9, 11)           # loads issued by the scalar (ACT) engine
ACT_ROWS = (0, 2, 0, 2, 3, 3, 2, 2, 3, 3, 0, 0)  # rows from tile front on ACT
GPS_TILES = ()                         # GpSimd TT concurrently with DVE packed
                                       # ops degrades DVE 5x (HW-traced); unused
IN_BUFS = 9
F16 = mybir.dt.float16
F32 = mybir.dt.float32

_nc_cache = []


def _dve_tree(nc, t, r0, r1, h1, h2, h3, s_all, lo):
    """Row sums of t[:, r0:r1, :1024] -> s_all[:, lo:lo+(r1-r0)] on DVE."""
    n = r1 - r0
    if n == 1:
        nc.vector.reduce_sum(
            out=s_all[:, lo : lo + 1], in_=t[:, r0, :], axis=mybir.AxisListType.X
        )
        return
    nc.vector.tensor_add(
        out=h1[:, 0:n, :], in0=t[:, r0:r1, 0:512], in1=t[:, r0:r1, 512:1024]
    )
    nc.vector.tensor_add(
        out=h2[:, 0:n, :], in0=h1[:, 0:n, 0:256], in1=h1[:, 0:n, 256:512]
    )
    nc.vector.tensor_add(
        out=h3[:, 0:n, :], in0=h2[:, 0:n, 0:128], in1=h2[:, 0:n, 128:256]
    )
    nc.vector.reduce_sum(
        out=s_all[:, lo : lo + n], in_=h3[:, 0:n, :], axis=mybir.AxisListType.X
    )


def _build():
    assert sum(R_SCHED) == ROWS_PER_PART
    nc = bacc.Bacc()
    x = nc.declare_dram_parameter("x", [ROWS, FEAT], F16, isOutput=False)
    y = nc.declare_dram_parameter("y", [P, ROWS_PER_PART], F32, isOutput=True)
    xv = x[:, :].rearrange("(p n) d -> p n d", p=P)

    max_r = max(R_SCHED)
    max_act = 4

    with tile.TileContext(nc) as tc:
        with (
            tc.tile_pool(name="inp", bufs=IN_BUFS) as inp,
            tc.tile_pool(name="sums", bufs=1) as sums_pool,
            tc.tile_pool(name="tree", bufs=1) as tree_pool,
            tc.tile_pool(name="scr", bufs=1) as scr_pool,
        ):
            s_all = sums_pool.tile([P, ROWS_PER_PART], F32, tag="s")
            h1 = tree_pool.tile([P, max_r, 512], F16, tag="h1")
            h2 = tree_pool.tile([P, max_r, 256], F16, tag="h2")
            h3 = tree_pool.tile([P, max_r, 128], F16, tag="h3")
            scr = scr_pool.tile([P, max_act, FEAT], F16, tag="scr")

            # --- all loads first (ring issues precede any compute in
            # each sequencer's FIFO)
            tiles, rows_of = [], []
            row = 0
            for i, r in enumerate(R_SCHED):
                t = inp.tile([P, r, FEAT], F16, tag="in")
                eng = nc.scalar if i in RING_B else nc.sync
                eng.dma_start(out=t[:, :, :], in_=xv[:, row : row + r, :])
                tiles.append(t)
                rows_of.append(row)
                row += r

            # --- ACT rows (front rows of its tiles, ascending)
            for i, r in enumerate(R_SCHED):
                for j in range(ACT_ROWS[i]):
                    nc.scalar.activation(
                        out=scr[:, j % max_act, :],
                        in_=tiles[i][:, j, :],
                        func=mybir.ActivationFunctionType.Copy,
                        accum_out=s_all[:, rows_of[i] + j : rows_of[i] + j + 1],
                    )

            # --- DVE trees (ascending; tail tiles land last anyway)
            for i, r in enumerate(R_SCHED):
                a = ACT_ROWS[i]
                if r - a > 0:
                    _dve_tree(nc, tiles[i], a, r, h1, h2, h3, s_all, rows_of[i] + a)

            nc.scalar.dma_start(out=y[:, :], in_=s_all[:, :])
    nc.finalize()
    return nc


def _get_nc():
    if not _nc_cache:
        _nc_cache.append(_build())
    return _nc_cache[0]


def kernel(x: np.ndarray) -> np.ndarray:
    nc = _get_nc()
    xh = np.ascontiguousarray(np.asarray(x)).astype(np.float16)
    shards = np.split(xh, N_CORES, axis=0)
    in_maps = [{"x": s} for s in shards]
    res = run_bass_kernel_spmd(nc, in_maps, list(range(N_CORES)))
    sums = np.concatenate(
        [res.results[i]["y"].reshape(ROWS) for i in range(N_CORES)], axis=0
    )
    return np.broadcast_to(sums[:, None], (BATCH, FEAT))


# revision 12
# speedup vs baseline: 2.8560x; 1.1684x over previous
"""KAN layer (identity edges) Trainium2 kernel.

output[b, o] = sum_i x[b, i]  for all o  -- row-sum broadcast to (B, 1024).

Structural optimizations over a naive full-output kernel:

1. Rank-1 output: the device computes ONLY the row sums; the host
   reconstructs the broadcast as a stride-0 view during unshard (the
   reference's own jnp.broadcast_to is the same free view).  Removes the
   entire 256 MB output write from HBM.

2. fp16 ingest: the row-sum tolerates quantization easily (1024-term
   sum, fp32 accumulation; measured L2 rel err ~5e-4 vs the 2e-2 gate),
   so the host casts x to fp16 before upload, halving HBM read traffic
   to 16 MiB/core.

3. Three compute engines keep pace with the ~450 GB/s load stream
   (HW-probed rates):
     - DVE: within-row halving tensor_tensor adds (fp16 2x_1P packed
       mode, 2 elem/cycle) down to 128 wide + 1x reduce_sum (f32 out).
     - ACT: activation(Copy) with f32 accum_out, ~1.4 us/row, on the
       front rows of most tiles (~20 of 64 rows).
   GpSimd is deliberately NOT used: its tensor ops running concurrently
   with DVE packed ops degrade DVE ~5x (HW-traced SBUF interference).

4. Schedule: small ramp tiles (2,2,4,4) so the first data lands ~3 us
   after DMA start and compute begins immediately; ACT's load issues
   (none -- see RING_B) can never delay its compute
   ops in its FIFO; tail tiles are 2 rows so the post-load tail is
   ~3 us.
"""

import numpy as np

import concourse.tile as tile
from concourse import bacc, mybir
from concourse.bass_utils import run_bass_kernel_spmd

N_CORES = 8
BATCH = 65536
FEAT = 1024
ROWS = BATCH // N_CORES        # 8192 rows per core
P = 128                        # SBUF partitions
ROWS_PER_PART = ROWS // P      # 64 consecutive rows owned by each partition

R_SCHED = (2, 2, 4, 4, 8, 8, 8, 8, 8, 8, 2, 2)
RING_B = ()                            # ALL loads on the sync ring: issuing from
                                       # the scalar engine entangles load issues
                                       # with ACT compute in its FIFO (HW-traced
                                       # 5-8us load stalls); one HWDGE queue
                                       # sustains ~425 GB/s anyway
ACT_ROWS = (0, 2, 0, 2, 3, 3, 2, 2, 3, 3, 0, 0)  # rows from tile front on ACT
GPS_TILES = ()                         # GpSimd TT concurrently with DVE packed
                                       # ops degrades DVE 5x (HW-traced); unused
IN_BUFS = 10
F16 = mybir.dt.float16
F32 = mybir.dt.float32

_nc_cache = []


def _dve_tree(nc, t, r0, r1, h1, h2, h3, s_all, lo):
    """Row sums of t[:, r0:r1, :1024] -> s_all[:, lo:lo+(r1-r0)] on DVE."""
    n = r1 - r0
    if n == 1:
        nc.vector.reduce_sum(
            out=s_all[:, lo : lo + 1], in_=t[:, r0, :], axis=mybir.AxisListType.X
        )
        return
    nc.vector.tensor_add(
        out=h1[:, 0:n, :], in0=t[:, r0:r1, 0:512], in1=t[:, r0:r1, 512:1024]
    )
    nc.vector.tensor_add(
        out=h2[:, 0:n, :], in0=h1[:, 0:n, 0:256], in1=h1[:, 0:n, 256:512]
    )
    nc.vector.tensor_add(
        out=h3[:, 0:n, :], in0=h2[:, 0:n, 0:128], in1=h2[:, 0:n, 128:256]
    )
    nc.vector.reduce_sum(
        out=s_all[:, lo : lo + n], in_=h3[:, 0:n, :], axis=mybir.AxisListType.X
    )


def _build():
    assert sum(R_SCHED) == ROWS_PER_PART
    nc = bacc.Bacc()
    x = nc.declare_dram_parameter("x", [ROWS, FEAT], F16, isOutput=False)
    y = nc.declare_dram_parameter("y", [P, ROWS_PER_PART], F32, isOutput=True)
    xv = x[:, :].rearrange("(p n) d -> p n d", p=P)

    max_r = max(R_SCHED)
    max_act = 4

    with tile.TileContext(nc) as tc:
        with (
            tc.tile_pool(name="inp", bufs=IN_BUFS) as inp,
            tc.tile_pool(name="sums", bufs=1) as sums_pool,
            tc.tile_pool(name="tree", bufs=1) as tree_pool,
            tc.tile_pool(name="scr", bufs=1) as scr_pool,
        ):
            s_all = sums_pool.tile([P, ROWS_PER_PART], F32, tag="s")
            h1 = tree_pool.tile([P, max_r, 512], F16, tag="h1")
            h2 = tree_pool.tile([P, max_r, 256], F16, tag="h2")
            h3 = tree_pool.tile([P, max_r, 128], F16, tag="h3")
            scr = scr_pool.tile([P, max_act, FEAT], F16, tag="scr")

            # --- all loads first (ring issues precede any compute in
            # each sequencer's FIFO)
            tiles, rows_of = [], []
            row = 0
            for i, r in enumerate(R_SCHED):
                t = inp.tile([P, r, FEAT], F16, tag="in")
                eng = nc.scalar if i in RING_B else nc.sync
                eng.dma_start(out=t[:, :, :], in_=xv[:, row : row + r, :])
                tiles.append(t)
                rows_of.append(row)
                row += r

            # --- ACT rows (front rows of its tiles, ascending)
            for i, r in enumerate(R_SCHED):
                for j in range(ACT_ROWS[i]):
                    nc.scalar.activation(
                        out=scr[:, j % max_act, :],
                        in_=tiles[i][:, j, :],
                        func=mybir.ActivationFunctionType.Copy,
                        accum_out=s_all[:, rows_of[i] + j : rows_of[i] + j + 1],
                    )

            # --- DVE trees (ascending; tail tiles land last anyway)
            for i, r in enumerate(R_SCHED):
                a = ACT_ROWS[i]
                if r - a > 0:
                    _dve_tree(nc, tiles[i], a, r, h1, h2, h3, s_all, rows_of[i] + a)

            nc.sync.dma_start(out=y[:, :], in_=s_all[:, :])
    nc.finalize()
    return nc


def _get_nc():
    if not _nc_cache:
        _nc_cache.append(_build())
    return _nc_cache[0]


def kernel(x: np.ndarray) -> np.ndarray:
    nc = _get_nc()
    xh = np.ascontiguousarray(np.asarray(x)).astype(np.float16)
    shards = np.split(xh, N_CORES, axis=0)
    in_maps = [{"x": s} for s in shards]
    res = run_bass_kernel_spmd(nc, in_maps, list(range(N_CORES)))
    sums = np.concatenate(
        [res.results[i]["y"].reshape(ROWS) for i in range(N_CORES)], axis=0
    )
    return np.broadcast_to(sums[:, None], (BATCH, FEAT))


# revision 13
# speedup vs baseline: 2.9319x; 1.0266x over previous
"""KAN layer (identity edges) Trainium2 kernel.

output[b, o] = sum_i x[b, i]  for all o  -- row-sum broadcast to (B, 1024).

Structural optimizations over a naive full-output kernel:

1. Rank-1 output: the device computes ONLY the row sums; the host
   reconstructs the broadcast as a stride-0 view during unshard (the
   reference's own jnp.broadcast_to is the same free view).  Removes the
   entire 256 MB output write from HBM.

2. fp16 ingest: the row-sum tolerates quantization easily (1024-term
   sum, fp32 accumulation; measured L2 rel err ~5e-4 vs the 2e-2 gate),
   so the host casts x to fp16 before upload, halving HBM read traffic
   to 16 MiB/core.

3. Three compute engines keep pace with the ~450 GB/s load stream
   (HW-probed rates):
     - DVE: within-row halving tensor_tensor adds (fp16 2x_1P packed
       mode, 2 elem/cycle) down to 128 wide + 1x reduce_sum (f32 out).
     - ACT: activation(Copy) with f32 accum_out, ~1.4 us/row, on the
       front rows of most tiles (~20 of 64 rows).
   GpSimd is deliberately NOT used: its tensor ops running concurrently
   with DVE packed ops degrade DVE ~5x (HW-traced SBUF interference).

4. Schedule: small ramp tiles (2,2,4,4) so the first data lands ~3 us
   after DMA start and compute begins immediately; ACT's load issues
   (none -- see RING_B) can never delay its compute
   ops in its FIFO; tail tiles are 2 rows so the post-load tail is
   ~3 us.
"""

import numpy as np

import concourse.tile as tile
from concourse import bacc, mybir
from concourse.bass_utils import run_bass_kernel_spmd

N_CORES = 8
BATCH = 65536
FEAT = 1024
ROWS = BATCH // N_CORES        # 8192 rows per core
P = 128                        # SBUF partitions
ROWS_PER_PART = ROWS // P      # 64 consecutive rows owned by each partition

R_SCHED = (2, 2, 4, 4, 8, 8, 8, 8, 8, 8, 3, 1)
RING_B = ()                            # ALL loads on the sync ring: issuing from
                                       # the scalar engine entangles load issues
                                       # with ACT compute in its FIFO (HW-traced
                                       # 5-8us load stalls); one HWDGE queue
                                       # sustains ~425 GB/s anyway
ACT_ROWS = (0, 2, 0, 2, 3, 3, 2, 2, 3, 3, 0, 1)  # rows from tile front on ACT
GPS_TILES = ()                         # GpSimd TT concurrently with DVE packed
                                       # ops degrades DVE 5x (HW-traced); unused
IN_BUFS = 10
F16 = mybir.dt.float16
F32 = mybir.dt.float32

_nc_cache = []


def _dve_tree(nc, t, r0, r1, h1, h2, h3, s_all, lo):
    """Row sums of t[:, r0:r1, :1024] -> s_all[:, lo:lo+(r1-r0)] on DVE."""
    n = r1 - r0
    if n == 1:
        nc.vector.reduce_sum(
            out=s_all[:, lo : lo + 1], in_=t[:, r0, :], axis=mybir.AxisListType.X
        )
        return
    nc.vector.tensor_add(
        out=h1[:, 0:n, :], in0=t[:, r0:r1, 0:512], in1=t[:, r0:r1, 512:1024]
    )
    nc.vector.tensor_add(
        out=h2[:, 0:n, :], in0=h1[:, 0:n, 0:256], in1=h1[:, 0:n, 256:512]
    )
    nc.vector.tensor_add(
        out=h3[:, 0:n, :], in0=h2[:, 0:n, 0:128], in1=h2[:, 0:n, 128:256]
    )
    nc.vector.reduce_sum(
        out=s_all[:, lo : lo + n], in_=h3[:, 0:n, :], axis=mybir.AxisListType.X
    )


def _build():
    assert sum(R_SCHED) == ROWS_PER_PART
    nc = bacc.Bacc()
    x = nc.declare_dram_parameter("x", [ROWS, FEAT], F16, isOutput=False)
    y = nc.declare_dram_parameter("y", [P, ROWS_PER_PART], F32, isOutput=True)
    xv = x[:, :].rearrange("(p n) d -> p n d", p=P)

    max_r = max(R_SCHED)
    max_act = 4

    with tile.TileContext(nc) as tc:
        with (
            tc.tile_pool(name="inp", bufs=IN_BUFS) as inp,
            tc.tile_pool(name="sums", bufs=1) as sums_pool,
            tc.tile_pool(name="tree", bufs=1) as tree_pool,
            tc.tile_pool(name="scr", bufs=1) as scr_pool,
        ):
            s_all = sums_pool.tile([P, ROWS_PER_PART], F32, tag="s")
            h1 = tree_pool.tile([P, max_r, 512], F16, tag="h1")
            h2 = tree_pool.tile([P, max_r, 256], F16, tag="h2")
            h3 = tree_pool.tile([P, max_r, 128], F16, tag="h3")
            scr = scr_pool.tile([P, max_act, FEAT], F16, tag="scr")

            # --- all loads first (ring issues precede any compute in
            # each sequencer's FIFO)
            tiles, rows_of = [], []
            row = 0
            for i, r in enumerate(R_SCHED):
                t = inp.tile([P, r, FEAT], F16, tag="in")
                eng = nc.scalar if i in RING_B else nc.sync
                eng.dma_start(out=t[:, :, :], in_=xv[:, row : row + r, :])
                tiles.append(t)
                rows_of.append(row)
                row += r

            # --- ACT rows (front rows of its tiles, ascending)
            for i, r in enumerate(R_SCHED):
                for j in range(ACT_ROWS[i]):
                    nc.scalar.activation(
                        out=scr[:, j % max_act, :],
                        in_=tiles[i][:, j, :],
                        func=mybir.ActivationFunctionType.Copy,
                        accum_out=s_all[:, rows_of[i] + j : rows_of[i] + j + 1],
                    )

            # --- DVE trees (ascending; tail tiles land last anyway)
            for i, r in enumerate(R_SCHED):
                a = ACT_ROWS[i]
                if r - a > 0:
                    _dve_tree(nc, tiles[i], a, r, h1, h2, h3, s_all, rows_of[i] + a)

            # split store: rows 0..59 (tiles 0-9) go out as soon as their
            # reduces land; only the 4 tail-tile sums trail the last reduce
            nc.sync.dma_start(out=y[:, 0:60], in_=s_all[:, 0:60])
            nc.sync.dma_start(out=y[:, 60:64], in_=s_all[:, 60:64])
    nc.finalize()
    return nc


def _get_nc():
    if not _nc_cache:
        _nc_cache.append(_build())
    return _nc_cache[0]


def kernel(x: np.ndarray) -> np.ndarray:
    nc = _get_nc()
    xh = np.ascontiguousarray(np.asarray(x)).astype(np.float16)
    shards = np.split(xh, N_CORES, axis=0)
    in_maps = [{"x": s} for s in shards]
    res = run_bass_kernel_spmd(nc, in_maps, list(range(N_CORES)))
    sums = np.concatenate(
        [res.results[i]["y"].reshape(ROWS) for i in range(N_CORES)], axis=0
    )
    return np.broadcast_to(sums[:, None], (BATCH, FEAT))
